# revision 1
# baseline (speedup 1.0000x reference)
"""DecomposedMaSA Trainium2 kernel.

Full inputs -> shard batch B=8 over 8 NeuronCores (1 image per core) ->
Bass/Tile kernel per core -> gather.

Per-core algorithm (B=1, C=128, H=W=128, heads=4, d=32), all channel-major:
  x [C, H*W] -> Q, K (channel-major, scale folded into Wq), VT1/VT2
  (spatial-major V tiles for the H-axis / W-axis attention).
  Per (axis, row): scores S[z,j] via 4 head-row-tiled matmuls; E=exp(S) on
  ACT (batched over 4 rows); Etilde = E * decay (DVE, bf16); softmax sums
  s[j] via ones-matmuls (col-tiled per head); s broadcast over partitions
  via K=1 matmuls; 1/s via DVE fast reciprocal; out tile = VT^T @ Etilde
  (col-tiled per head); y_pad <- out * (1/s) fused into the PSUM->SBUF copy.
  LePE dwconv3x3 + 1x1 proj fused as 9 shifted matmuls with host-computed
  M_tap = w_proj @ diag(w_dw[:, tap]).
"""

import numpy as np
from contextlib import ExitStack

import concourse.bass as bass
import concourse.tile as tile
from concourse import mybir, bacc
from concourse.bass_utils import run_bass_kernel_spmd

F32 = mybir.dt.float32
BF16 = mybir.dt.bfloat16

C = 128
HW = 128  # H == W == 128
P = 128  # partitions
NHEADS = 4
DHEAD = 32
NPIX = HW * HW  # 16384
PAD = HW + 2  # 130
NPAD = PAD * PAD  # 16900

_BUILD_CACHE = {}


def build_kernel(stage="full"):
    nc = bacc.Bacc("TRN2", target_bir_lowering=False, debug=False)

    x_d = nc.dram_tensor("x", [P, NPIX], F32, kind="ExternalInput")
    wq_d = nc.dram_tensor("wq_t", [P, C], F32, kind="ExternalInput")
    wk_d = nc.dram_tensor("wk_t", [P, C], F32, kind="ExternalInput")
    wv_d = nc.dram_tensor("wv_t", [P, C], F32, kind="ExternalInput")
    d16_d = nc.dram_tensor("d16", [P, 4 * 512], F32, kind="ExternalInput")
    mt_d = nc.dram_tensor("mtaps", [P, 9 * C], F32, kind="ExternalInput")
    out_d = nc.dram_tensor("out", [P, NPIX], F32, kind="ExternalOutput")

    with tile.TileContext(nc) as tc:
        with ExitStack() as ctx:
            _body(ctx, tc, nc, x_d, wq_d, wk_d, wv_d, d16_d, mt_d, out_d, stage)
    nc.compile()
    return nc


def _body(ctx, tc, nc, x_d, wq_d, wk_d, wv_d, d16_d, mt_d, out_d, stage="full"):
    Exp = mybir.ActivationFunctionType.Exp

    const_pool = ctx.enter_context(tc.tile_pool(name="consts", bufs=1))
    big_pool = ctx.enter_context(tc.tile_pool(name="big", bufs=1))

    # ---- constants in SBUF (bf16 for matmul operands) ----
    wq_sb = const_pool.tile([P, C], BF16, tag="wq")
    wk_sb = const_pool.tile([P, C], BF16, tag="wk")
    wv_sb = const_pool.tile([P, C], BF16, tag="wv")
    d16_sb = const_pool.tile([P, 4 * 512], BF16, tag="d16")
    mt_sb = const_pool.tile([P, 9 * C], BF16, tag="mt")
    ones_sb = const_pool.tile([P, 32], BF16, tag="ones")
    nc.gpsimd.dma_start(wq_sb[:], wq_d[:])
    nc.gpsimd.dma_start(wk_sb[:], wk_d[:])
    nc.gpsimd.dma_start(wv_sb[:], wv_d[:])
    nc.gpsimd.dma_start(d16_sb[:], d16_d[:])
    nc.gpsimd.dma_start(mt_sb[:], mt_d[:])
    nc.gpsimd.memset(ones_sb[:], 1.0)

    # ---- x load (bf16 cast during DMA) ----
    x_sb = big_pool.tile([P, NPIX], BF16, tag="x")
    for i in range(4):
        nc.gpsimd.dma_start(
            x_sb[:, i * 4096 : (i + 1) * 4096], x_d[:, i * 4096 : (i + 1) * 4096]
        )

    # ---- projections: Q, K channel-major ----
    q_sb = big_pool.tile([P, NPIX], BF16, tag="q")
    k_sb = big_pool.tile([P, NPIX], BF16, tag="k")
    vt1_sb = big_pool.tile([P, NPIX], BF16, tag="vt1")
    vt2_sb = big_pool.tile([P, NPIX], BF16, tag="vt2")
    with tc.tile_pool(name="proj_ps", bufs=4, space="PSUM") as proj_ps:
        for j, (w_t, dst) in enumerate(((wq_sb, q_sb), (wk_sb, k_sb))):
            for ch in range(32):
                ps = proj_ps.tile([P, 512], F32, tag="ps")
                nc.tensor.matmul(ps[:], w_t[:], x_sb[:, ch * 512 : (ch + 1) * 512])
                nc.vector.tensor_copy(dst[:, ch * 512 : (ch + 1) * 512], ps[:])

        # ---- VT1 / VT2: spatial-major V tiles ----
        # VT1 tile t: [z, c], z = W-position inside row t  (x cols t*128 .. +128)
        # VT2 tile t: [z, c], z = H-position at column t   (x cols t, t+128, ...)
        x_r = x_sb[:].rearrange("p (t w) -> p w t", w=HW)  # elem = x[p, t*128+w]
        for t in range(HW):
            for vt_dst, lhs in (
                (vt1_sb, x_sb[:, t * HW : (t + 1) * HW]),
                (vt2_sb, x_r[:, t, :]),
            ):
                ps = proj_ps.tile([P, HW], F32, tag="psv")
                nc.tensor.matmul(ps[:], lhs, wv_sb[:])
                nc.vector.tensor_copy(vt_dst[:, t * HW : (t + 1) * HW], ps[:])

    if stage == "proj":
        nc.gpsimd.dma_start(out_d[:, 0:NPIX], q_sb[:])
        return

    # ---- attention (both axes) ----
    # y_pad reuses x's slot (tag="x"): x's last read is the VT2 matmuls above.
    y_pad = big_pool.tile([P, NPAD], BF16, tag="x")
    nc.gpsimd.memset(y_pad[:], 0.0)

    y_pr = y_pad[:].rearrange("p (r c) -> p r c", c=PAD)

    with (
        tc.tile_pool(name="qk_ps", bufs=1, space="PSUM") as qk_ps,
        tc.tile_pool(name="stat_ps", bufs=1, space="PSUM") as stat_ps,
        tc.tile_pool(name="av_ps", bufs=2, space="PSUM") as av_ps,
        tc.tile_pool(name="e_pool", bufs=2) as e_pool,
        tc.tile_pool(name="small", bufs=3) as small_pool,
    ):
        _attention(tc, nc, q_sb, k_sb, vt1_sb, vt2_sb, d16_sb, ones_sb, y_pr,
                   qk_ps, stat_ps, av_ps, e_pool, small_pool)

    if stage == "attn":
        nc.gpsimd.dma_start(out_d[:, 0:NPIX], y_pad[:, 0:NPIX])
        return

    # ---- fused dwconv3x3 + proj: out = sum_tap Mtap @ y_pad(shifted) ----
    with (
        tc.tile_pool(name="f_ps", bufs=4, space="PSUM") as f_ps,
        tc.tile_pool(name="o_pool", bufs=4) as o_pool,
    ):
        for ch in range(32):  # 4 rows of the image per chunk
            r0 = ch * 4
            ps_f = f_ps.tile([P, 512], F32, tag="f")
            t = 0
            for dy in range(3):
                for dx in range(3):
                    nc.tensor.matmul(
                        ps_f[:],
                        mt_sb[:, t * C : (t + 1) * C],
                        y_pr[:, r0 + dy : r0 + dy + 4, dx : dx + HW],
                        start=(t == 0),
                        stop=(t == 8),
                    )
                    t += 1
            o_sb = o_pool.tile([P, 512], F32, tag="o")
            nc.scalar.copy(o_sb[:], ps_f[:])
            nc.gpsimd.dma_start(out_d[:, r0 * HW : (r0 + 4) * HW], o_sb[:])


def _attention(tc, nc, q_sb, k_sb, vt1_sb, vt2_sb, d16_sb, ones_sb, y_pr,
               qk_ps, stat_ps, av_ps, e_pool, small_pool):
    import os
    sub = os.environ.get("ATTN_SUB", "full")
    naxes = 1 if "ax0" in sub else 2
    Exp = mybir.ActivationFunctionType.Exp
    for axis in range(naxes):
        if axis == 0:
            q_t = q_sb[:].rearrange("p (t j) -> p t j", j=HW)
            k_t = k_sb[:].rearrange("p (t j) -> p t j", j=HW)
            vt_sb = vt1_sb
        else:
            q_t = q_sb[:].rearrange("p (t j) -> p j t", j=HW)
            k_t = k_sb[:].rearrange("p (t j) -> p j t", j=HW)
            vt_sb = vt2_sb

        for b in range(32):  # batches of 4 row-units
            ps_qk = qk_ps.tile([P, 2048], F32, tag="qk")
            for u in range(4):
                i = b * 4 + u
                for h in range(4):
                    hp = h * DHEAD
                    nc.tensor.matmul(
                        ps_qk[:, h * 512 + u * HW : h * 512 + (u + 1) * HW],
                        k_t[hp : hp + DHEAD, i, :],
                        q_t[hp : hp + DHEAD, i, :],
                        tile_position=(hp, 0),
                    )
            e_sb = e_pool.tile([P, 2048], BF16, tag="e")
            nc.scalar.activation(e_sb[:], ps_qk[:], Exp)
            if sub == "qkexp":
                if b == 31:
                    nc.gpsimd.dma_start(y_pr[:, 0:8, 0:PAD], e_sb[:, 0:8*PAD])
                continue
            et_sb = e_pool.tile([P, 2048], BF16, tag="et")
            nc.vector.tensor_mul(et_sb[:], e_sb[:], d16_sb[:])

            # softmax sums: s_h[j] at partition 32h, cols u*128..
            skip_norm = "nonorm" in sub
            ps_stat = stat_ps.tile([P, 512], F32, tag="stat")
            if not skip_norm:
              for u in range(4):
                for h in range(4):
                    hp = h * DHEAD
                    nc.tensor.matmul(
                        ps_stat[hp : hp + 1, u * HW : (u + 1) * HW],
                        ones_sb[:, 0:1],
                        e_sb[:, h * 512 + u * HW : h * 512 + (u + 1) * HW],
                        tile_position=(0, hp),
                    )
            srow_sb = small_pool.tile([P, 512], BF16, tag="srow")
            if not skip_norm:
                nc.scalar.copy(srow_sb[:], ps_stat[:])

            # broadcast s over the 32 partitions of each head block
            ps_bc = stat_ps.tile([P, 512], F32, tag="bc")
            for h in range(4 if not skip_norm else 0):
                hp = h * DHEAD
                nc.tensor.matmul(
                    ps_bc[hp : hp + DHEAD, :],
                    ones_sb[hp : hp + 1, 0:DHEAD],
                    srow_sb[hp : hp + 1, :],
                    tile_position=(hp, hp),
                )
            rs_sb = small_pool.tile([P, 512], F32, tag="rs")
            if not skip_norm:
                nc.vector.reciprocal_approx_fast(rs_sb[:], ps_bc[:])
            else:
                nc.gpsimd.memset(rs_sb[:], 1.0)

            # attention @ V (col-tiled per head) -> channel-major out tiles
            ps_av = av_ps.tile([P, 512], F32, tag="av")
            for u in range(4):
                i = b * 4 + u
                for h in range(4):
                    hp = h * DHEAD
                    nc.tensor.matmul(
                        ps_av[hp : hp + DHEAD, u * HW : (u + 1) * HW],
                        vt_sb[:, i * HW + hp : i * HW + hp + DHEAD],
                        et_sb[:, h * 512 + u * HW : h * 512 + (u + 1) * HW],
                        tile_position=(0, hp),
                    )

            # renormalize + write into padded y
            for u in range(4):
                i = b * 4 + u
                if axis == 0:
                    dst = y_pr[:, i + 1, 1 : HW + 1]
                    nc.vector.tensor_mul(
                        dst, ps_av[:, u * HW : (u + 1) * HW], rs_sb[:, u * HW : (u + 1) * HW]
                    )
                else:
                    tmp = small_pool.tile([P, HW], BF16, tag="tmp")
                    nc.vector.tensor_mul(
                        tmp[:], ps_av[:, u * HW : (u + 1) * HW], rs_sb[:, u * HW : (u + 1) * HW]
                    )
                    dst = y_pr[:, 1 : HW + 1, i + 1]
                    nc.vector.tensor_add(dst, dst, tmp[:])


def _host_prep(x, w_qkv, w_proj, w_dw, gamma):
    B = x.shape[0]
    scale = DHEAD ** -0.5
    wq_t = np.ascontiguousarray((w_qkv[0:C] * scale).T).astype(np.float32)
    wk_t = np.ascontiguousarray(w_qkv[C : 2 * C].T).astype(np.float32)
    wv_t = np.ascontiguousarray(w_qkv[2 * C : 3 * C].T).astype(np.float32)
    idx = np.arange(HW, dtype=np.float32)
    dmat = np.exp(-float(gamma) * np.abs(idx[:, None] - idx[None, :])).astype(np.float32)
    d16 = np.tile(np.concatenate([dmat] * 4, axis=1), (1, 4))  # [128, 2048]
    # M_tap = w_proj @ diag(w_dw[:, 0, dy, dx]); lhsT layout -> M_tap.T [c, o]
    mtaps = np.empty((P, 9 * C), dtype=np.float32)
    t = 0
    for dy in range(3):
        for dx in range(3):
            m = w_proj * w_dw[:, 0, dy, dx][None, :]  # [o, c] * diag over c
            mtaps[:, t * C : (t + 1) * C] = m.T
            t += 1
    xs = x.reshape(B, C, NPIX).astype(np.float32)
    return xs, wq_t, wk_t, wv_t, d16.astype(np.float32), mtaps


def kernel(x, w_qkv, w_proj, w_dw, gamma):
    x = np.asarray(x, dtype=np.float32)
    w_qkv = np.asarray(w_qkv, dtype=np.float32)
    w_proj = np.asarray(w_proj, dtype=np.float32)
    w_dw = np.asarray(w_dw, dtype=np.float32)
    gamma = np.float32(gamma)

    B = x.shape[0]
    xs, wq_t, wk_t, wv_t, d16, mtaps = _host_prep(x, w_qkv, w_proj, w_dw, gamma)

    if "nc" not in _BUILD_CACHE:
        _BUILD_CACHE["nc"] = build_kernel()
    nc = _BUILD_CACHE["nc"]

    in_maps = [
        {
            "x": np.ascontiguousarray(xs[b]),
            "wq_t": wq_t,
            "wk_t": wk_t,
            "wv_t": wv_t,
            "d16": d16,
            "mtaps": mtaps,
        }
        for b in range(B)
    ]
    res = run_bass_kernel_spmd(nc, in_maps, core_ids=list(range(8)))
    out = np.stack([res.results[b]["out"].reshape(C, HW, HW) for b in range(B)])
    return out.astype(np.float32)



# revision 13
# speedup vs baseline: 1.6299x; 1.6299x over previous
"""DecomposedMaSA Trainium2 kernel (v2).

Full inputs -> shard batch B=8 over 8 NeuronCores (1 image per core) ->
Bass/Tile kernel per core -> gather.

Per-core algorithm (B=1, C=128, H=W=128, heads=4, d=32):
  x [C, H*W] bf16 -> Q, K channel-major (scale folded into Wq), VT1/VT2
  spatial-major V tiles.
  Attention per (axis, unit of 4 rows/cols):
    scores via 16 row-tiled matmuls into two [128,1024] PSUM halves;
    E = exp(S) on ACT (two ACTIVATEs, PSUM->SBUF bf16);
    Et = E * decay on DVE (one [128,2048] bf16 2x op);
    softmax sums+broadcast fused: per head one matmul with all-ones
    [128,32] stationary -> s replicated on the 32-partition head strip;
    1/s via DVE fast reciprocal; AV via 16 col-tiled matmuls;
    renorm fused into PSUM->SBUF eviction with 3D-AP writes into the
    padded y buffer (axis1 = W-axis first: strided-col write; axis0
    second: mul to tmp + 2x-mode in-place add).
  LePE dwconv3x3 + 1x1 proj fused as 9 shifted matmuls with
  M_tap = w_proj @ diag(w_dw[:, tap]); interleaved into the axis0 phase.
  y_pad geometry: 130 rows x 132 cols, image (r, j) -> y_pad[r+1, j+2]
  (left pad 2 keeps row starts 4B-aligned for DVE 2x adds).
"""

import numpy as np
from contextlib import ExitStack

import concourse.bass as bass
import concourse.tile as tile
from concourse import mybir, bacc
from concourse.bass_utils import run_bass_kernel_spmd

F32 = mybir.dt.float32
BF16 = mybir.dt.bfloat16

C = 128
HW = 128  # H == W == 128
P = 128  # partitions
NHEADS = 4
DHEAD = 32
NPIX = HW * HW  # 16384
PADR = HW + 2  # 130 rows
PADC = HW + 4  # 132 cols
NPAD = PADR * PADC  # 17160

_BUILD_CACHE = {}


def build_kernel(stage="full"):
    nc = bacc.Bacc("TRN2", target_bir_lowering=False, debug=False)

    x_d = nc.dram_tensor("x", [P, NPIX], F32, kind="ExternalInput")
    wq_d = nc.dram_tensor("wq_t", [P, C], F32, kind="ExternalInput")
    wk_d = nc.dram_tensor("wk_t", [P, C], F32, kind="ExternalInput")
    wv_d = nc.dram_tensor("wv_t", [P, C], F32, kind="ExternalInput")
    d16_d = nc.dram_tensor("d16", [P, 4 * 512], F32, kind="ExternalInput")
    mt_d = nc.dram_tensor("mtaps", [P, 9 * C], F32, kind="ExternalInput")
    out_d = nc.dram_tensor("out", [P, NPIX], F32, kind="ExternalOutput")

    with tile.TileContext(nc) as tc:
        with ExitStack() as ctx:
            _body(ctx, tc, nc, x_d, wq_d, wk_d, wv_d, d16_d, mt_d, out_d, stage)
    nc.compile()
    return nc


def _body(ctx, tc, nc, x_d, wq_d, wk_d, wv_d, d16_d, mt_d, out_d, stage="full"):
    const_pool = ctx.enter_context(tc.tile_pool(name="consts", bufs=1))
    big_pool = ctx.enter_context(tc.tile_pool(name="big", bufs=1))

    # ---- constants in SBUF (bf16 for matmul operands) ----
    wq_sb = const_pool.tile([P, C], BF16, tag="wq")
    wk_sb = const_pool.tile([P, C], BF16, tag="wk")
    wv_sb = const_pool.tile([P, C], BF16, tag="wv")
    d16_sb = const_pool.tile([P, 4 * 512], BF16, tag="d16")
    mt_sb = const_pool.tile([P, 9 * C], BF16, tag="mt")
    ones_sb = const_pool.tile([P, DHEAD], BF16, tag="ones")
    nc.gpsimd.dma_start(wq_sb[:], wq_d[:])
    nc.gpsimd.dma_start(wk_sb[:], wk_d[:])
    nc.gpsimd.dma_start(wv_sb[:], wv_d[:])
    nc.gpsimd.dma_start(d16_sb[:], d16_d[:])
    nc.gpsimd.dma_start(mt_sb[:], mt_d[:])
    nc.gpsimd.memset(ones_sb[:], 1.0)

    # ---- x load (bf16 cast during DMA), 8 chunks for pipelining ----
    x_sb = big_pool.tile([P, NPIX], BF16, tag="x")
    for i in range(8):
        nc.gpsimd.dma_start(
            x_sb[:, i * 2048 : (i + 1) * 2048], x_d[:, i * 2048 : (i + 1) * 2048]
        )

    # ---- projections ----
    q_sb = big_pool.tile([P, NPIX], BF16, tag="q")
    k_sb = big_pool.tile([P, NPIX], BF16, tag="k")
    vt1_sb = big_pool.tile([P, NPIX], BF16, tag="vt1")
    vt2_sb = big_pool.tile([P, NPIX], BF16, tag="vt2")
    ncopy = 0
    with tc.tile_pool(name="proj_ps", bufs=4, space="PSUM") as proj_ps:
        # Q, K channel-major: weight stationary, x moving, N=512
        for w_t, dst in ((wq_sb, q_sb), (wk_sb, k_sb)):
            for ch in range(32):
                ps = proj_ps.tile([P, 512], F32, tag="ps")
                nc.tensor.matmul(ps[:], w_t[:], x_sb[:, ch * 512 : (ch + 1) * 512])
                dsl = dst[:, ch * 512 : (ch + 1) * 512]
                if ncopy % 3 == 2:
                    nc.scalar.copy(dsl, ps[:])
                else:
                    nc.vector.tensor_copy(dsl, ps[:])
                ncopy += 1

        # VT1 / VT2: spatial-major V tiles, groups of 4 into one bank
        # VT1 tile t: [z, c], z = W-position inside row t
        # VT2 tile t: [z, c], z = H-position at column t
        x_r = x_sb[:].rearrange("p (t w) -> p w t", w=HW)  # elem = x[p, t*128+w]
        for vt_dst, is_col in ((vt1_sb, False), (vt2_sb, True)):
            for g in range(32):
                ps = proj_ps.tile([P, 512], F32, tag="psv")
                for k in range(4):
                    t = g * 4 + k
                    lhs = x_r[:, t, :] if is_col else x_sb[:, t * HW : (t + 1) * HW]
                    nc.tensor.matmul(ps[:, k * HW : (k + 1) * HW], lhs, wv_sb[:])
                dsl = vt_dst[:, g * 512 : (g + 1) * 512]
                if ncopy % 3 == 2:
                    nc.scalar.copy(dsl, ps[:])
                else:
                    nc.vector.tensor_copy(dsl, ps[:])
                ncopy += 1

    if stage == "proj":
        nc.gpsimd.dma_start(out_d[:, 0:NPIX], q_sb[:])
        return

    # ---- padded output accumulator (reuses x's SBUF slot) ----
    y_pad = big_pool.tile([P, NPAD], BF16, tag="x")
    nc.gpsimd.memset(y_pad[:], 0.0)
    y_pr = y_pad[:].rearrange("p (r c) -> p r c", c=PADC)  # [p, row, col]
    y_pc = y_pad[:].rearrange("p (r c) -> p c r", c=PADC)  # [p, col, row]

    Exp = mybir.ActivationFunctionType.Exp

    with (
        tc.tile_pool(name="e_pool", bufs=2) as e_pool,
        tc.tile_pool(name="small", bufs=2) as small_pool,
    ):
        # ================= axis 1 (W-axis attention) first =================
        with (
            tc.tile_pool(name="qk1_ps", bufs=1, space="PSUM") as qk_ps,
            tc.tile_pool(name="stat1_ps", bufs=2, space="PSUM") as stat_ps,
            tc.tile_pool(name="av1_ps", bufs=2, space="PSUM") as av_ps,
        ):
            _axis_attention(
                tc, nc, 1, q_sb, k_sb, vt2_sb, d16_sb, ones_sb, y_pr, y_pc,
                qk_ps, stat_ps, av_ps, e_pool, small_pool, None, None, None, None,
            )

        if stage == "attn1":
            nc.gpsimd.dma_start(out_d[:, 0:NPIX], y_pad[:, 0:NPIX])
            return

        # ========== axis 0 (H-axis attention) + interleaved LePE ==========
        with (
            tc.tile_pool(name="qk0_ps", bufs=1, space="PSUM") as qk_ps,
            tc.tile_pool(name="stat0_ps", bufs=1, space="PSUM") as stat_ps,
            tc.tile_pool(name="av0_ps", bufs=1, space="PSUM") as av_ps,
            tc.tile_pool(name="f_ps", bufs=2, space="PSUM") as f_ps,
            tc.tile_pool(name="o_pool", bufs=3) as o_pool,
        ):
            _axis_attention(
                tc, nc, 0, q_sb, k_sb, vt1_sb, d16_sb, ones_sb, y_pr, y_pc,
                qk_ps, stat_ps, av_ps, e_pool, small_pool,
                mt_sb, f_ps, o_pool, out_d,
            )


def _axis_attention(tc, nc, axis, q_sb, k_sb, vt_sb, d16_sb, ones_sb, y_pr, y_pc,
                    qk_ps, stat_ps, av_ps, e_pool, small_pool,
                    mt_sb, f_ps, o_pool, out_d):
    """One attention axis over 32 units of 4 rows (axis0) / 4 cols (axis1),
    software-pipelined: unit i's scores+exp are emitted one step ahead of
    unit i-1's sums/AV/renorm. axis0 also interleaves LePE chunks."""
    import os
    sub = os.environ.get("ATTN_SUB", "full")
    Exp = mybir.ActivationFunctionType.Exp
    if axis == 0:
        q_t = q_sb[:].rearrange("p (t j) -> p t j", j=HW)
        k_t = k_sb[:].rearrange("p (t j) -> p t j", j=HW)
    else:
        q_t = q_sb[:].rearrange("p (t j) -> p j t", j=HW)
        k_t = k_sb[:].rearrange("p (t j) -> p j t", j=HW)

    NU = 32
    prev = None  # state of unit i-1: (e, et, stat, rs, av, i)
    lepe_done = 0

    for i in range(NU + 1):
        if i < NU:
            # ---- scores for unit i: two PSUM tiles, one per HEAD PAIR ----
            # (concurrent row-tiled head MMs must drain into different
            # banks: heads h, h+1 -> the tile's two banks)
            # e layout: cols = h*512 + u*128 + j (contiguous per exp)
            e_sb = e_pool.tile([P, 2048], BF16, tag="e")
            for half in range(2):
                ps_h = qk_ps.tile([P, 1024], F32, tag=f"qk{half}")
                for u in range(4):
                    r = i * 4 + u
                    for hh in range(2):
                        h = half * 2 + hh
                        hp = h * DHEAD
                        nc.tensor.matmul(
                            ps_h[:, hh * 512 + u * HW : hh * 512 + (u + 1) * HW],
                            k_t[hp : hp + DHEAD, r, :],
                            q_t[hp : hp + DHEAD, r, :],
                            tile_position=(hp, 0),
                        )
                # exp: PSUM f32 -> SBUF bf16, contiguous
                nc.scalar.activation(
                    e_sb[:, half * 1024 : (half + 1) * 1024], ps_h[:], Exp
                )
            cur_e = e_sb

        if prev is not None and sub != "qkexp":
            (e_p, i_p) = prev
            # ---- decay multiply (DVE, one bf16 2x op) ----
            et_sb = e_pool.tile([P, 2048], BF16, tag="et")
            nc.vector.tensor_mul(et_sb[:], e_p[:], d16_sb[:])

            # ---- softmax sums + broadcast: one matmul per head ----
            ps_stat = stat_ps.tile([P, 512], F32, tag="stat")
            rs_sb = small_pool.tile([P, 512], F32, tag="rs")
            if sub in ("full", "sums"):
                for h in range(4):
                    hp = h * DHEAD
                    nc.tensor.matmul(
                        ps_stat[hp : hp + DHEAD, :],
                        ones_sb[:, 0:DHEAD],
                        e_p[:, h * 512 : (h + 1) * 512],
                        tile_position=(0, hp),
                    )
                nc.vector.reciprocal_approx_fast(rs_sb[:], ps_stat[:])
            else:
                nc.gpsimd.memset(rs_sb[:], 1.0)

            if sub in ("et", "sums"):
                prev = (cur_e, i) if i < NU else None
                continue
            # ---- attention @ V (col-tiled per head) ----
            ps_av = av_ps.tile([P, 512], F32, tag="av")
            for u in range(4):
                r = i_p * 4 + u
                for h in range(4):
                    hp = h * DHEAD
                    nc.tensor.matmul(
                        ps_av[hp : hp + DHEAD, u * HW : (u + 1) * HW],
                        vt_sb[:, r * HW + hp : r * HW + hp + DHEAD],
                        et_sb[:, h * 512 + u * HW : h * 512 + (u + 1) * HW],
                        tile_position=(0, hp),
                    )

            # ---- renorm + write into padded y ----
            av_r = ps_av[:].rearrange("p (u j) -> p u j", j=HW)
            rs_r = rs_sb[:].rearrange("p (u j) -> p u j", j=HW)
            if axis == 1:
                dst = y_pc[:, i_p * 4 + 2 : i_p * 4 + 6, 1 : HW + 1]
                nc.vector.tensor_mul(dst, av_r, rs_r)
            else:
                tmp = small_pool.tile([P, 512], BF16, tag="tmp")
                nc.vector.tensor_mul(tmp[:], ps_av[:], rs_sb[:])
                dst = y_pr[:, i_p * 4 + 1 : i_p * 4 + 5, 2 : HW + 2]
                nc.vector.tensor_add(
                    dst, dst, tmp[:].rearrange("p (u j) -> p u j", j=HW)
                )

        # ---- interleaved LePE (axis0 only): chunk c ready once units
        # <= c+1 have renormed; stay 2 units behind ----
        if axis == 0 and mt_sb is not None:
            while lepe_done < NU and lepe_done + 2 <= i - 1:
                _lepe_chunk(nc, f_ps, o_pool, mt_sb, y_pr, out_d, lepe_done)
                lepe_done += 1

        prev = (cur_e, i) if i < NU else None

    if axis == 0 and mt_sb is not None:
        while lepe_done < NU:
            _lepe_chunk(nc, f_ps, o_pool, mt_sb, y_pr, out_d, lepe_done)
            lepe_done += 1


def _lepe_chunk(nc, f_ps, o_pool, mt_sb, y_pr, out_d, c):
    """Fused dwconv3x3 + 1x1 proj for 4 image rows starting at 4c."""
    r0 = c * 4
    ps_f = f_ps.tile([P, 512], F32, tag="f")
    t = 0
    for dy in range(3):
        for dx in range(3):
            nc.tensor.matmul(
                ps_f[:],
                mt_sb[:, t * C : (t + 1) * C],
                y_pr[:, r0 + dy : r0 + dy + 4, 1 + dx : 1 + dx + HW],
                start=(t == 0),
                stop=(t == 8),
            )
            t += 1
    o_sb = o_pool.tile([P, 512], F32, tag="o")
    nc.scalar.copy(o_sb[:], ps_f[:])
    nc.gpsimd.dma_start(out_d[:, r0 * HW : (r0 + 4) * HW], o_sb[:])


def _host_prep(x, w_qkv, w_proj, w_dw, gamma):
    B = x.shape[0]
    scale = DHEAD ** -0.5
    wq_t = np.ascontiguousarray((w_qkv[0:C] * scale).T).astype(np.float32)
    wk_t = np.ascontiguousarray(w_qkv[C : 2 * C].T).astype(np.float32)
    wv_t = np.ascontiguousarray(w_qkv[2 * C : 3 * C].T).astype(np.float32)
    idx = np.arange(HW, dtype=np.float32)
    dmat = np.exp(-float(gamma) * np.abs(idx[:, None] - idx[None, :])).astype(np.float32)
    d16 = np.tile(np.concatenate([dmat] * 4, axis=1), (1, 4))  # [128, 2048]
    # M_tap = w_proj @ diag(w_dw[:, 0, dy, dx]); lhsT layout -> M_tap.T [c, o]
    mtaps = np.empty((P, 9 * C), dtype=np.float32)
    t = 0
    for dy in range(3):
        for dx in range(3):
            m = w_proj * w_dw[:, 0, dy, dx][None, :]  # [o, c] * diag over c
            mtaps[:, t * C : (t + 1) * C] = m.T
            t += 1
    xs = x.reshape(B, C, NPIX).astype(np.float32)
    return xs, wq_t, wk_t, wv_t, d16.astype(np.float32), mtaps


def kernel(x, w_qkv, w_proj, w_dw, gamma):
    x = np.asarray(x, dtype=np.float32)
    w_qkv = np.asarray(w_qkv, dtype=np.float32)
    w_proj = np.asarray(w_proj, dtype=np.float32)
    w_dw = np.asarray(w_dw, dtype=np.float32)
    gamma = np.float32(gamma)

    B = x.shape[0]
    xs, wq_t, wk_t, wv_t, d16, mtaps = _host_prep(x, w_qkv, w_proj, w_dw, gamma)

    if "nc" not in _BUILD_CACHE:
        _BUILD_CACHE["nc"] = build_kernel()
    nc = _BUILD_CACHE["nc"]

    in_maps = [
        {
            "x": np.ascontiguousarray(xs[b]),
            "wq_t": wq_t,
            "wk_t": wk_t,
            "wv_t": wv_t,
            "d16": d16,
            "mtaps": mtaps,
        }
        for b in range(B)
    ]
    res = run_bass_kernel_spmd(nc, in_maps, core_ids=list(range(8)))
    out = np.stack([res.results[b]["out"].reshape(C, HW, HW) for b in range(B)])
    return out.astype(np.float32)
